# revision 16
# baseline (speedup 1.0000x reference)
"""DMPNN message-passing kernel for 8 Trainium2 NeuronCores (Bass/Tile).

Strategy (edge/data parallel per the sharding hint), v2 — bf16 pipeline:
  - Edges come in reverse pairs (2k, 2k+1); pairs are sharded across the 8
    cores.  rev(e) is the sibling array position (hE/hO split), so the
    DMPNN reverse-edge subtraction is a plain elementwise op.
  - All state is bf16.  Node tables are split into LO (32768 rows) / HI
    halves so every dma_gather / dma_scatter_add index fits int16.
  - agg gathers use dma_gather(transpose=True), which lands node rows
    feature-major in SBUF directly — no PE transposes on the input side.
  - h state lives feature-major in DRAM ([128, NP] per direction), so the
    per-step reload is a plain contiguous DMA.
  - Compute is feature-major: h' = relu(Wu^T (A - h_rev) + h_self) with a
    stationary Wu; biases are folded into the gather tables host-side
    (agg tables are initialized to Wu^-T bu / 8, node tables carry
    Wi_A^-T bi etc.), so no bias ops exist on device.
  - Only the scatter input needs edge-major tiles: PE transposes (matmul
    against identity) + scalar-engine PSUM->SBUF copies produce them.
  - segment_sum = local bf16 dma_scatter_add + AllReduce.  The AllReduce
    is split LO/HI: AR(LO) issues after segments {0,1,2} and overlaps
    segment 3; AR(HI) issues after segment 3 and overlaps the next pass's
    segment 0 (which only reads LO).
"""
import sys

sys.path.insert(0, "/opt/trn_rl_repo")

import numpy as np

N_CORES = 8
D = 128
DE = 32
STEPS = 4
GROUP = 2048        # pairs per group
CHUNK = 512         # fm matmul chunk (one PSUM bank)
UNROLL = 2
HB = 63 * 512       # node half boundary (rows 0..HB-1 in LO table)
PAD_HI = 512
LOTAB = HB + PAD_HI  # 32768 rows; int16 indices 0..32767


def _ceil(x, m):
    return (x + m - 1) // m * m


def _solve_bias(WT, b):
    """c with WT @ c == b (exact for b == 0)."""
    if not np.any(b):
        return np.zeros(WT.shape[1], np.float32)
    return np.linalg.lstsq(WT.astype(np.float64), b.astype(np.float64),
                           rcond=None)[0].astype(np.float32)


def _window_assign(s, d, group, node_max, max_win=192):
    """Assign each pair to a window of size `group` such that within every
    window all d values are distinct and all s values are distinct (the
    dma_scatter_add engine-race constraint).  Greedy rounds, vectorized."""
    n = s.size
    win = np.full(n, -1, np.int32)
    used_s = np.zeros((node_max, max_win), bool)
    used_d = np.zeros((node_max, max_win), bool)
    full = np.zeros(max_win, bool)
    cnt = np.zeros(max_win, np.int64)
    rem = np.arange(n)
    while rem.size:
        free = ~(used_s[s[rem]] | used_d[d[rem]] | full[None, :])
        assert free.any(axis=1).all(), "window assigner ran out of windows"
        w = np.argmax(free, axis=1).astype(np.int64)
        order = np.lexsort((rem, w))
        ws, rs = w[order], rem[order]
        ds_, ss_ = d[rs], s[rs]
        kd = ws * np.int64(node_max) + ds_
        ks = ws * np.int64(node_max) + ss_
        first_d = np.zeros(ws.size, bool)
        first_s = np.zeros(ws.size, bool)
        od = np.lexsort((np.arange(ws.size), kd))
        first_d[od[np.concatenate(([True], kd[od][1:] != kd[od][:-1]))]] = True
        os_ = np.lexsort((np.arange(ws.size), ks))
        first_s[os_[np.concatenate(([True], ks[os_][1:] != ks[os_][:-1]))]] = True
        uw, st, cts = np.unique(ws, return_index=True, return_counts=True)
        rank = np.arange(ws.size) - np.repeat(st, cts)
        ok = first_d & first_s & (rank < np.repeat(group - cnt[uw], cts))
        acc = rs[ok]
        wacc = ws[ok]
        win[acc] = wacc
        used_d[d[acc], wacc] = True
        used_s[s[acc], wacc] = True
        np.add.at(cnt, wacc, 1)
        full = cnt >= group
        rem = rem[win[rem] < 0]
    return win, (int(cnt.nonzero()[0].max()) + 1) if n else 0


def _wrap16(v, bf=None):
    """int16 index array -> [128, n/16] wrapped+replicated layout."""
    t = np.asarray(v, np.int16).reshape(-1, 16).T
    return np.ascontiguousarray(np.tile(t, (8, 1)))


def _prep(node_feature, edge_feature, edge_src, edge_dst,
          Wi, bi, Wu, bu, Wf, bf,
          n_cores=N_CORES, group=GROUP, unroll=UNROLL, hb=HB):
    import ml_dtypes

    BF = ml_dtypes.bfloat16
    node_feature = np.asarray(node_feature, np.float32)
    edge_feature = np.asarray(edge_feature, np.float32)
    Wi = np.asarray(Wi, np.float32)
    Wu = np.asarray(Wu, np.float32)
    Wf = np.asarray(Wf, np.float32)
    bi = np.asarray(bi, np.float32)
    bu = np.asarray(bu, np.float32)
    bf = np.asarray(bf, np.float32)
    edge_src = np.asarray(edge_src)
    edge_dst = np.asarray(edge_dst)
    N = node_feature.shape[0]
    E = edge_src.shape[0]
    P = E // 2
    assert P % n_cores == 0
    per = P // n_cores

    s_all = edge_src[0::2].astype(np.int64)
    d_all = edge_dst[0::2].astype(np.int64)
    efE_all = edge_feature[0::2]
    efO_all = edge_feature[1::2]

    HI_N = N - hb
    LO_TRASH = hb
    HI_TRASH = HI_N
    OUT_PAD = _ceil(N, CHUNK)
    HITAB = _ceil(max(HI_N + 1, OUT_PAD - hb + 1), 2048)

    # bias folds (exact when biases are zero)
    ci = _solve_bias(Wi[:D].T, bi)                       # init gather table
    cu = _solve_bias(Wu.T, bu)                           # agg tables
    cf = _solve_bias(Wf[:D].T, bf - Wf[D:].T @ cu)       # final nf table

    def split_tab(rows_lo, rows_hi):
        lo = np.zeros((LOTAB, D), BF)
        hi = np.zeros((HITAB, D), BF)
        lo[0:hb] = rows_lo
        hi[0:HI_N] = rows_hi
        return lo, hi

    nfiL, nfiH = split_tab(node_feature[0:hb] + ci, node_feature[hb:N] + ci)
    nffL = np.zeros((D, LOTAB), BF)
    nffH = np.zeros((D, HITAB), BF)
    nffL[:, 0:hb] = (node_feature[0:hb] + cf).T
    nffH[:, 0:HI_N] = (node_feature[hb:N] + cf).T
    cu8 = np.broadcast_to((cu / n_cores).astype(BF), (128, 16, D))
    cu8 = np.ascontiguousarray(cu8.reshape(128, 16 * D))
    fin_idx = _wrap16(np.arange(LOTAB, dtype=np.int64))

    # Per (core, segment): window assignment (distinct s & d per window).
    cores = []
    nwin = np.zeros((n_cores, 4), np.int64)
    for c in range(n_cores):
        sl = slice(c * per, (c + 1) * per)
        sc, dc = s_all[sl], d_all[sl]
        a = (dc >= hb).astype(np.int64)
        b = (sc >= hb).astype(np.int64)
        seg = a * 2 + b
        per_seg = []
        for g in range(4):
            m = np.flatnonzero(seg == g)
            s_m, d_m = sc[m], dc[m]
            degd = np.bincount(d_m, minlength=N)
            degs = np.bincount(s_m, minlength=N)
            prio = np.argsort(-(degd[d_m] + degs[s_m]), kind="stable")
            win_p, nw = _window_assign(s_m[prio], d_m[prio], group, N)
            win = np.empty_like(win_p)
            win[prio] = win_p
            key = np.lexsort((s_m, win))
            per_seg.append((m[key], win[key], nw))
            nwin[c, g] = nw
        cores.append((sc, dc, efE_all[sl], efO_all[sl], per_seg))
    gchunk = group * unroll
    seg_nw = [int(_ceil(max(int(nwin[:, g].max()), 1) * group, gchunk)) // group
              for g in range(4)]
    seg_sz = [nw * group for nw in seg_nw]
    NP_ = int(sum(seg_sz))
    seg_start = [0, seg_sz[0], seg_sz[0] + seg_sz[1],
                 seg_sz[0] + seg_sz[1] + seg_sz[2]]

    shards = []
    for c in range(n_cores):
        sc, dc, efE_c, efO_c, per_seg = cores[c]
        sIdx = np.zeros(NP_, np.int64)
        dIdx = np.zeros(NP_, np.int64)
        ef2 = np.zeros((DE, 2 * NP_), BF)
        for g in range(4):
            a, b = g // 2, g % 2
            st = seg_start[g]
            s_tr = LO_TRASH if b == 0 else HI_TRASH
            d_tr = LO_TRASH if a == 0 else HI_TRASH
            sIdx[st:st + seg_sz[g]] = s_tr
            dIdx[st:st + seg_sz[g]] = d_tr
            order, wins, nw = per_seg[g]
            if order.size:
                counts = np.bincount(wins, minlength=nw)
                assert counts.max() <= group
                starts = np.concatenate(([0], np.cumsum(counts)))[:-1]
                rank = np.arange(order.size) - starts[wins]
                pos = st + wins * group + rank
                sIdx[pos] = sc[order] - hb * b
                dIdx[pos] = dc[order] - hb * a
                ef2[:, pos] = efE_c[order].T
                ef2[:, NP_ + pos] = efO_c[order].T
        assert sIdx.max() < 32768 and dIdx.max() < 32768

        # combined per-group idx layout: [s_g (128 cols) | d_g (128 cols)]
        ng = NP_ // group
        gsIdx = np.zeros((128, ng * 2 * (group // 16)), np.int16)
        w = group // 16
        for g in range(ng):
            gsIdx[:, g * 2 * w:(g * 2 + 1) * w] = \
                _wrap16(sIdx[g * group:(g + 1) * group])
            gsIdx[:, (g * 2 + 1) * w:(g * 2 + 2) * w] = \
                _wrap16(dIdx[g * group:(g + 1) * group])

        shards.append({
            "nfiL": nfiL, "nfiH": nfiH, "nffL": nffL, "nffH": nffH,
            "ef2": np.ascontiguousarray(ef2),
            "gsIdx": np.ascontiguousarray(gsIdx),
            "finIdx": fin_idx, "cu8": cu8,
        })

    meta = dict(N=N, NP=NP_, LOTAB=LOTAB, HITAB=HITAB, OUT_PAD=OUT_PAD,
                seg_sz=seg_sz, seg_start=seg_start, HB=hb,
                n_cores=n_cores, group=group, unroll=unroll)
    return shards, meta


def _build(meta):
    import os as _os

    import concourse.bass as bass
    import concourse.tile as tile
    from concourse import bacc, mybir

    f32 = mybir.dt.float32
    bf16 = mybir.dt.bfloat16
    i16 = mybir.dt.int16
    NP_ = meta["NP"]
    LOT = meta["LOTAB"]
    HIT = meta["HITAB"]
    OUT_PAD = meta["OUT_PAD"]
    group = meta["group"]
    unroll = meta["unroll"]
    n_cores = meta["n_cores"]
    hb = meta["HB"]
    jb = hb // CHUNK            # final-pass LO slab count
    GW = group // 16            # idx cols per half-group
    NBLK = group // 128         # 16 em blocks per group-half... per 2048 pairs
    NCH = 2 * group // CHUNK    # fm chunks per group (E cols + O cols)

    nc = bacc.Bacc("TRN2", target_bir_lowering=False, debug=False,
                   enable_asserts=False, num_devices=n_cores)

    nfiL_t = nc.dram_tensor("nfiL", [LOT, D], bf16, kind="ExternalInput")
    nfiH_t = nc.dram_tensor("nfiH", [HIT, D], bf16, kind="ExternalInput")
    nffL_t = nc.dram_tensor("nffL", [D, LOT], bf16, kind="ExternalInput")
    nffH_t = nc.dram_tensor("nffH", [D, HIT], bf16, kind="ExternalInput")
    ef2_t = nc.dram_tensor("ef2", [DE, 2 * NP_], bf16, kind="ExternalInput")
    gsIdx_t = nc.dram_tensor("gsIdx", [128, NP_ // 16 * 2], i16,
                             kind="ExternalInput")
    finIdx_t = nc.dram_tensor("finIdx", [128, LOT // 16], i16,
                              kind="ExternalInput")
    cu8_t = nc.dram_tensor("cu8", [128, 16 * D], bf16, kind="ExternalInput")
    WiA_t = nc.dram_tensor("WiA", [D, D], bf16, kind="ExternalInput")
    WiB_t = nc.dram_tensor("WiB", [DE, D], bf16, kind="ExternalInput")
    Wu_t = nc.dram_tensor("Wu", [D, D], bf16, kind="ExternalInput")
    WfA_t = nc.dram_tensor("WfA", [D, D], bf16, kind="ExternalInput")
    WfB_t = nc.dram_tensor("WfB", [D, D], bf16, kind="ExternalInput")
    id_t = nc.dram_tensor("ident", [D, D], bf16, kind="ExternalInput")
    out_t = nc.dram_tensor("out", [OUT_PAD, D], f32, kind="ExternalOutput")

    with tile.TileContext(nc) as tc:
        with (
            tc.tile_pool(name="const", bufs=1) as constp,
            tc.tile_pool(name="work", bufs=2) as work,
            tc.tile_pool(name="psF", bufs=3, space="PSUM") as psF,
            tc.tile_pool(name="psE", bufs=3, space="PSUM") as psE,
            tc.tile_pool(name="dram", bufs=1, space="DRAM") as dram,
        ):
            def const_load(name, shape, dt, src_ap):
                t = constp.tile(shape, dt, tag=name, name=name)
                nc.sync.dma_start(t[:], src_ap)
                return t

            WiA = const_load("WiA", [D, D], bf16, WiA_t.ap())
            WiB = const_load("WiB", [DE, D], bf16, WiB_t.ap())
            Wu_sb = const_load("Wu", [D, D], bf16, Wu_t.ap())
            WfA = const_load("WfA", [D, D], bf16, WfA_t.ap())
            WfB = const_load("WfB", [D, D], bf16, WfB_t.ap())
            id_sb = const_load("ident", [D, D], bf16, id_t.ap())
            cu8_sb = const_load("cu8", [128, 16 * D], bf16, cu8_t.ap())
            gsIdx_sb = constp.tile([128, NP_ // 16 * 2], i16, tag="gsIdx",
                                   name="gsIdx_sb")
            nc.sync.dma_start(gsIdx_sb[:], gsIdx_t.ap())
            finIdx_sb = constp.tile([128, LOT // 16], i16, tag="finIdx",
                                    name="finIdx_sb")
            nc.sync.dma_start(finIdx_sb[:], finIdx_t.ap())

            # ---- DRAM state ----
            hE = [dram.tile([D, NP_], bf16, name=f"hE{k}", tag=f"hE{k}")
                  for k in range(2)]
            hO = [dram.tile([D, NP_], bf16, name=f"hO{k}", tag=f"hO{k}")
                  for k in range(2)]
            LL = [dram.tile([LOT, D], bf16, name=f"aggLL{k}", tag=f"aggLL{k}")
                  for k in range(2)]
            LH = [dram.tile([HIT, D], bf16, name=f"aggLH{k}", tag=f"aggLH{k}")
                  for k in range(2)]
            SL = [dram.tile([LOT, D], bf16, name=f"aggSL{k}", tag=f"aggSL{k}",
                            addr_space="Shared") for k in range(STEPS + 1)]
            SH = [dram.tile([HIT, D], bf16, name=f"aggSH{k}", tag=f"aggSH{k}",
                            addr_space="Shared") for k in range(STEPS + 1)]

            def fill_table(t, rows):
                zr = cu8_sb[:].rearrange("p (a f) -> p a f", f=D)
                r0 = 0
                while r0 < rows:
                    nc.sync.dma_start(
                        t[:][r0:r0 + 2048, :]
                        .rearrange("(a p) f -> p a f", p=128),
                        zr)
                    r0 += 2048

            def idx_cols(gg, which):
                # which: 0 = s half, 1 = d half, 2 = both
                if which == 2:
                    return gsIdx_sb[:, bass.ds(gg * 2 * GW, 2 * GW)]
                return gsIdx_sb[:, bass.ds((gg * 2 + which) * GW, GW)]

            def tab(t):
                return t[:] if hasattr(t, "opt") else t.ap()

            def body(kind, seg, gg, col0, gsrc, gtgt, hin, hout):
                """One group of `group` pairs.  gg = global group index,
                col0 = pair column base.  gsrc/gtgt: (LO, HI) table pairs."""
                a, b = seg // 2, seg % 2
                A = work.tile([128, 1, 2 * group], bf16, tag="A", name="A")
                if a == b:
                    nc.gpsimd.dma_gather(
                        A[:], tab(gsrc[b]), idx_cols(gg, 2),
                        num_idxs=2 * group, num_idxs_reg=2 * group,
                        elem_size=D, transpose=True, single_packet=False)
                else:
                    nc.gpsimd.dma_gather(
                        A[:, :, 0:group], tab(gsrc[b]), idx_cols(gg, 0),
                        num_idxs=group, num_idxs_reg=group,
                        elem_size=D, transpose=True, single_packet=False)
                    nc.gpsimd.dma_gather(
                        A[:, :, group:2 * group], tab(gsrc[a]), idx_cols(gg, 1),
                        num_idxs=group, num_idxs_reg=group,
                        elem_size=D, transpose=True, single_packet=False)
                Av = A[:].rearrange("p a n -> p (a n)")

                if kind == "step":
                    hld = work.tile([128, 2, group], bf16, tag="hld",
                                    name="hld")
                    nc.sync.dma_start(hld[:, 0, :],
                                      hin[1][:][:, bass.ds(col0, group)])
                    nc.sync.dma_start(hld[:, 1, :],
                                      hin[0][:][:, bass.ds(col0, group)])
                    hldv = hld[:].rearrange("p a n -> p (a n)")
                    msg = work.tile([128, 2 * group], bf16, tag="msg",
                                    name="msg")
                    nc.vector.tensor_sub(msg[:], Av, hldv)
                else:
                    ef_sb = work.tile([DE, 2 * group], bf16, tag="ef",
                                      name="ef_sb")
                    nc.sync.dma_start(ef_sb[:, 0:group],
                                      ef2_t.ap()[:, bass.ds(col0, group)])
                    nc.sync.dma_start(
                        ef_sb[:, group:2 * group],
                        ef2_t.ap()[:, bass.ds(NP_ + col0, group)])

                hf = work.tile([128, 2 * group], bf16, tag="hf", name="hf")
                emE = work.tile([128, NBLK, D], bf16, tag="emE", name="emE")
                emO = work.tile([128, NBLK, D], bf16, tag="emO", name="emO")
                for c in range(NCH):
                    c0 = c * CHUNK
                    ps = psF.tile([128, CHUNK], f32, tag="fm", name="ps")
                    if kind == "step":
                        nc.tensor.matmul(ps[:], Wu_sb[:],
                                         msg[:, c0:c0 + CHUNK],
                                         start=True, stop=True)
                        self_sl = (hld[:, 1, c0:c0 + CHUNK] if c < NCH // 2
                                   else hld[:, 0, c0 - group:c0 - group + CHUNK])
                        tmp = work.tile([128, CHUNK], bf16, tag="tmp",
                                        name="tmp")
                        nc.vector.tensor_add(tmp[:], ps[:], self_sl)
                        nc.vector.tensor_relu(hf[:, c0:c0 + CHUNK], tmp[:])
                    else:
                        nc.tensor.matmul(ps[:], WiA[:], Av[:, c0:c0 + CHUNK],
                                         start=True, stop=False)
                        nc.tensor.matmul(ps[:], WiB[:],
                                         ef_sb[:, c0:c0 + CHUNK],
                                         start=False, stop=True)
                        nc.vector.tensor_relu(hf[:, c0:c0 + CHUNK], ps[:])
                    pse = psE.tile([128, CHUNK // D, D], bf16, tag="em",
                                   name="pse")
                    for t in range(CHUNK // D):
                        blk = c0 // D + t
                        nc.tensor.transpose(
                            pse[:, t, :],
                            hf[:, blk * D:(blk + 1) * D], id_sb[:])
                    em_t = emE if c < NCH // 2 else emO
                    eb = (c0 if c < NCH // 2 else c0 - group) // D
                    nc.scalar.copy(em_t[:, eb:eb + CHUNK // D, :], pse[:])

                if hout is not None:
                    nc.sync.dma_start(
                        hout[0][:][:, bass.ds(col0, group)], hf[:, 0:group])
                    nc.sync.dma_start(
                        hout[1][:][:, bass.ds(col0, group)],
                        hf[:, group:2 * group])
                # E states scatter by d into half `a`; O states by s into `b`
                nc.gpsimd.dma_scatter_add(
                    gtgt[a][:], emE[:], idx_cols(gg, 1),
                    num_idxs=group, num_idxs_reg=group, elem_size=D,
                    single_packet=False)
                nc.gpsimd.dma_scatter_add(
                    gtgt[b][:], emO[:], idx_cols(gg, 0),
                    num_idxs=group, num_idxs_reg=group, elem_size=D,
                    single_packet=False)

            def loop_i(lo, hi, step, body_fn):
                if _os.environ.get("BASS_SIM_UNROLL") == "1":
                    for i in range(lo, hi, step):
                        body_fn(i)
                else:
                    with tc.For_i(lo, hi, step) as i:
                        body_fn(i)

            def allreduce(src, dst):
                nc.gpsimd.collective_compute(
                    "AllReduce", mybir.AluOpType.add,
                    replica_groups=[list(range(n_cores))],
                    ins=[src.opt()], outs=[dst.opt()])

            def run_pass(p):
                kind = "init" if p == 0 else "step"
                par = p % 2
                gsrc = (SL[p - 1], SH[p - 1]) if p > 0 else (nfiL_t, nfiH_t)
                gtgt = (LL[par], LH[par])
                hin = (hE[p % 2], hO[p % 2]) if p > 0 else None
                hout = ((hE[(p + 1) % 2], hO[(p + 1) % 2])
                        if p < STEPS else None)
                if p < STEPS:   # zero next pass's scatter targets
                    fill_table(LL[1 - par], LOT)
                    fill_table(LH[1 - par], HIT)
                for seg in range(4):
                    n_groups = meta["seg_sz"][seg] // group
                    g0 = meta["seg_start"][seg] // group

                    def seg_body(i, seg=seg, g0=g0):
                        for j in range(unroll):
                            gg = g0 + i + j
                            body(kind, seg, gg, (g0 + i + j) * group,
                                 gsrc, gtgt, hin, hout)

                    loop_i(0, n_groups, unroll, seg_body)
                    if seg == 2:
                        allreduce(LL[par], SL[p])
                allreduce(LH[par], SH[p])

            fill_table(LL[0], LOT)
            fill_table(LH[0], HIT)
            for p in range(STEPS + 1):
                run_pass(p)

            # ---- final: out = relu([nf || agg] @ Wf + bf) ----------------
            aggF = (SL[STEPS], SH[STEPS])
            nff = (nffL_t, nffH_t)

            def final_body(j, hi, roff):
                nf_fm = work.tile([128, CHUNK], bf16, tag="fnf", name="nf_fm")
                nc.sync.dma_start(
                    nf_fm[:], nff[hi].ap()[:, bass.ds(j * CHUNK - roff, CHUNK)])
                ag_fm = work.tile([128, 1, CHUNK], bf16, tag="fag",
                                  name="ag_fm")
                nc.gpsimd.dma_gather(
                    ag_fm[:], aggF[hi][:],
                    finIdx_sb[:, bass.ds(j * (CHUNK // 16) - roff // 16,
                                         CHUNK // 16)],
                    num_idxs=CHUNK, num_idxs_reg=CHUNK, elem_size=D,
                    transpose=True, single_packet=False)
                pso = psE.tile([128, CHUNK // D, D], f32, tag="em",
                               name="pso")
                for t in range(CHUNK // D):
                    nc.tensor.matmul(pso[:, t, :],
                                     nf_fm[:, t * D:(t + 1) * D],
                                     WfA[:], start=True, stop=False)
                    nc.tensor.matmul(pso[:, t, :],
                                     ag_fm[:, 0, t * D:(t + 1) * D],
                                     WfB[:], start=False, stop=True)
                o_t = work.tile([128, CHUNK // D, D], f32, tag="fot",
                                name="o_t")
                nc.scalar.activation(o_t[:], pso[:],
                                     mybir.ActivationFunctionType.Relu)
                nc.sync.dma_start(
                    out_t.ap()[bass.ds(j * CHUNK, CHUNK), :]
                    .rearrange("(a p) f -> p a f", p=128),
                    o_t[:])

            loop_i(0, jb, 1, lambda j: final_body(j, 0, 0))
            loop_i(jb, OUT_PAD // CHUNK, 1, lambda j: final_body(j, 1, hb))

    nc.compile()
    return nc


LAST_RESULTS = None
LAST_TIMES = None


def _run_spmd(nc, in_maps, time_iters=0):
    """Execute the bass module on len(in_maps) axon cores via PJRT."""
    import time as _time

    import jax
    from jax.experimental.shard_map import shard_map
    from jax.sharding import Mesh, NamedSharding, PartitionSpec

    from concourse import bass2jax, mybir

    bass2jax.install_neuronx_cc_hook()
    n_cores = len(in_maps)
    partition_name = (nc.partition_id_tensor.name
                      if nc.partition_id_tensor else None)
    in_names, out_names, out_avals, zero_outs = [], [], [], []
    for alloc in nc.m.functions[0].allocations:
        if not isinstance(alloc, mybir.MemoryLocationSet):
            continue
        name = alloc.memorylocations[0].name
        if alloc.kind == "ExternalInput":
            if name != partition_name:
                in_names.append(name)
        elif alloc.kind == "ExternalOutput":
            shape = tuple(alloc.tensor_shape)
            dtype = mybir.dt.np(alloc.dtype)
            out_names.append(name)
            out_avals.append(jax.core.ShapedArray(shape, dtype))
            zero_outs.append(np.zeros(shape, dtype))
    n_params = len(in_names)
    full_in_names = list(in_names) + list(out_names)
    if partition_name is not None:
        full_in_names.append(partition_name)

    def _body(*args):
        operands = list(args)
        if partition_name is not None:
            operands.append(bass2jax.partition_id_tensor())
        outs = bass2jax._bass_exec_p.bind(
            *operands,
            out_avals=tuple(out_avals),
            in_names=tuple(full_in_names),
            out_names=tuple(out_names),
            lowering_input_output_aliases=(),
            sim_require_finite=True,
            sim_require_nnan=True,
            nc=nc,
        )
        return tuple(outs)

    devices = jax.devices()[:n_cores]
    mesh = Mesh(np.asarray(devices), ("core",))
    spec = NamedSharding(mesh, PartitionSpec("core"))
    n_in = n_params + len(zero_outs)
    fn = jax.jit(shard_map(_body, mesh=mesh,
                           in_specs=(PartitionSpec("core"),) * n_in,
                           out_specs=(PartitionSpec("core"),) * len(out_names),
                           check_rep=False))
    dev_in = [
        jax.device_put(
            np.concatenate([np.asarray(in_maps[c][k]) for c in range(n_cores)], 0),
            spec)
        for k in in_names
    ]
    dev_zero = [
        jax.device_put(np.zeros((n_cores * z.shape[0], *z.shape[1:]), z.dtype), spec)
        for z in zero_outs
    ]
    out = fn(*dev_in, *dev_zero)
    jax.block_until_ready(out)
    times = []
    for _ in range(time_iters):
        t0 = _time.perf_counter()
        out2 = fn(*dev_in, *dev_zero)
        jax.block_until_ready(out2)
        times.append(_time.perf_counter() - t0)
    results = [
        {name: np.asarray(out[i]).reshape(n_cores, *out_avals[i].shape)[c]
         for i, name in enumerate(out_names)}
        for c in range(n_cores)
    ]
    return results, times


def kernel(node_feature, edge_feature, edge_src, edge_dst,
           Wi, bi, Wu, bu, Wf, bf):
    import os

    import ml_dtypes

    BF = ml_dtypes.bfloat16
    global LAST_RESULTS, LAST_TIMES
    shards, meta = _prep(node_feature, edge_feature, edge_src, edge_dst,
                         Wi, bi, Wu, bu, Wf, bf)
    nc = _build(meta)

    Wi = np.asarray(Wi, np.float32)
    common = {
        "WiA": Wi[:D].astype(BF), "WiB": Wi[D:].astype(BF),
        "Wu": np.asarray(Wu, np.float32).astype(BF),
        "WfA": np.asarray(Wf, np.float32)[:D].astype(BF),
        "WfB": np.asarray(Wf, np.float32)[D:].astype(BF),
        "ident": np.eye(D, dtype=np.float32).astype(BF),
    }
    in_maps = [dict(sh, **common) for sh in shards]
    time_iters = int(os.environ.get("KERNEL_TIME_ITERS", "0"))
    results, times = _run_spmd(nc, in_maps, time_iters=time_iters)
    LAST_RESULTS = results
    LAST_TIMES = times
    return np.asarray(results[0]["out"][:meta["N"]], np.float32)


# revision 48
# speedup vs baseline: 1.1106x; 1.1106x over previous
"""DMPNN message-passing kernel for 8 Trainium2 NeuronCores (Bass/Tile).

Strategy (edge/data parallel per the sharding hint) — bf16 pipeline:
  - Edges come in reverse pairs (2k, 2k+1); pairs are sharded across the 8
    cores.  rev(e) is the sibling array position (hE/hO split), so the
    DMPNN reverse-edge subtraction is a plain elementwise op.
  - All state is bf16 (rel err ~7e-3 vs the fp32 reference).  Node tables
    are split into LO (32768 rows) / HI halves so every dma_gather /
    dma_scatter_add index fits int16.
  - agg gathers use dma_gather(transpose=True), which lands node rows
    feature-major in SBUF directly — no PE transposes on the input side.
  - h state lives feature-major in DRAM ([128, NP] per direction), so the
    per-step reload is a plain contiguous DMA.
  - Compute is feature-major: h' = relu(Wu^T (A - h_rev) + h_self) with a
    stationary Wu; biases are folded into the gather tables host-side
    (agg tables carry Wu^-T bu / 8, node tables carry Wi_A^-T bi etc.),
    so no bias ops exist on device.
  - Only the scatter input needs edge-major tiles: PE transposes (matmul
    against identity) + scalar-engine PSUM->SBUF copies produce them.
  - segment_sum = local dma_scatter_add into SBUF-RESIDENT parity-split
    tables (avoids HBM random-row read-modify-write), bulk-dumped to DRAM
    per half, then a bf16 AllReduce.  The AllReduce is split LO/HI:
    AR(LO) issues after segments {0,1,2} and overlaps segment 3; AR(HI)
    issues after segment 3 and overlaps the next pass's segment 0 (which
    only reads LO).
  - Loops are fully unrolled (For_i iterations carry an all-engine
    barrier); all SWDGE traffic stays on queue 0 — multi-queue completion
    semaphores are not ordering-safe under Tile.
"""
import sys

sys.path.insert(0, "/opt/trn_rl_repo")

import numpy as np

N_CORES = 8
D = 128
DE = 32
STEPS = 4
GROUP = 2048        # pairs per group
CHUNK = 512         # fm matmul chunk (one PSUM bank)
UNROLL = 2
HB = 63 * 512       # node half boundary (rows 0..HB-1 in LO table)
PAD_HI = 512
LOTAB = HB + PAD_HI  # 32768 rows; int16 indices 0..32767


def _ceil(x, m):
    return (x + m - 1) // m * m


def _solve_bias(WT, b):
    """c with WT @ c == b (exact for b == 0)."""
    if not np.any(b):
        return np.zeros(WT.shape[1], np.float32)
    return np.linalg.lstsq(WT.astype(np.float64), b.astype(np.float64),
                           rcond=None)[0].astype(np.float32)


def _window_assign(s, d, group, node_max, max_win=192):
    """Assign each pair to a window of size `group` such that within every
    window all d values are distinct and all s values are distinct (the
    dma_scatter_add engine-race constraint).  Greedy rounds, vectorized."""
    n = s.size
    win = np.full(n, -1, np.int32)
    used_s = np.zeros((node_max, max_win), bool)
    used_d = np.zeros((node_max, max_win), bool)
    full = np.zeros(max_win, bool)
    cnt = np.zeros(max_win, np.int64)
    rem = np.arange(n)
    while rem.size:
        free = ~(used_s[s[rem]] | used_d[d[rem]] | full[None, :])
        assert free.any(axis=1).all(), "window assigner ran out of windows"
        w = np.argmax(free, axis=1).astype(np.int64)
        order = np.lexsort((rem, w))
        ws, rs = w[order], rem[order]
        ds_, ss_ = d[rs], s[rs]
        kd = ws * np.int64(node_max) + ds_
        ks = ws * np.int64(node_max) + ss_
        first_d = np.zeros(ws.size, bool)
        first_s = np.zeros(ws.size, bool)
        od = np.lexsort((np.arange(ws.size), kd))
        first_d[od[np.concatenate(([True], kd[od][1:] != kd[od][:-1]))]] = True
        os_ = np.lexsort((np.arange(ws.size), ks))
        first_s[os_[np.concatenate(([True], ks[os_][1:] != ks[os_][:-1]))]] = True
        uw, st, cts = np.unique(ws, return_index=True, return_counts=True)
        rank = np.arange(ws.size) - np.repeat(st, cts)
        ok = first_d & first_s & (rank < np.repeat(group - cnt[uw], cts))
        acc = rs[ok]
        wacc = ws[ok]
        win[acc] = wacc
        used_d[d[acc], wacc] = True
        used_s[s[acc], wacc] = True
        np.add.at(cnt, wacc, 1)
        full = cnt >= group
        rem = rem[win[rem] < 0]
    return win, (int(cnt.nonzero()[0].max()) + 1) if n else 0


def _wrap16(v, bf=None):
    """int16 index array -> [128, n/16] wrapped+replicated layout."""
    t = np.asarray(v, np.int16).reshape(-1, 16).T
    return np.ascontiguousarray(np.tile(t, (8, 1)))


def _prep(node_feature, edge_feature, edge_src, edge_dst,
          Wi, bi, Wu, bu, Wf, bf,
          n_cores=N_CORES, group=GROUP, unroll=UNROLL, hb=HB):
    import ml_dtypes

    BF = ml_dtypes.bfloat16
    node_feature = np.asarray(node_feature, np.float32)
    edge_feature = np.asarray(edge_feature, np.float32)
    Wi = np.asarray(Wi, np.float32)
    Wu = np.asarray(Wu, np.float32)
    Wf = np.asarray(Wf, np.float32)
    bi = np.asarray(bi, np.float32)
    bu = np.asarray(bu, np.float32)
    bf = np.asarray(bf, np.float32)
    edge_src = np.asarray(edge_src)
    edge_dst = np.asarray(edge_dst)
    N = node_feature.shape[0]
    E = edge_src.shape[0]
    P = E // 2
    assert P % n_cores == 0
    per = P // n_cores

    s_all = edge_src[0::2].astype(np.int64)
    d_all = edge_dst[0::2].astype(np.int64)
    efE_all = edge_feature[0::2]
    efO_all = edge_feature[1::2]

    HI_N = N - hb
    LO_TRASH = hb
    HI_TRASH = HI_N
    OUT_PAD = _ceil(N, CHUNK)
    HITAB = _ceil(max(HI_N + 1, OUT_PAD - hb + 1), 2048)

    # bias folds (exact when biases are zero)
    ci = _solve_bias(Wi[:D].T, bi)                       # init gather table
    cu = _solve_bias(Wu.T, bu)                           # agg tables
    cf = _solve_bias(Wf[:D].T, bf - Wf[D:].T @ cu)       # final nf table

    def split_tab(rows_lo, rows_hi):
        lo = np.zeros((LOTAB, D), BF)
        hi = np.zeros((HITAB, D), BF)
        lo[0:hb] = rows_lo
        hi[0:HI_N] = rows_hi
        return lo, hi

    nfiL, nfiH = split_tab(node_feature[0:hb] + ci, node_feature[hb:N] + ci)
    nffL = np.zeros((D, LOTAB), BF)
    nffH = np.zeros((D, HITAB), BF)
    nffL[:, 0:hb] = (node_feature[0:hb] + cf).T
    nffH[:, 0:HI_N] = (node_feature[hb:N] + cf).T
    cu8 = np.broadcast_to((cu / n_cores).astype(BF), (128, 16, D))
    cu8 = np.ascontiguousarray(cu8.reshape(128, 16 * D))
    fin_idx = _wrap16(np.arange(LOTAB, dtype=np.int64))

    # Per (core, segment): window assignment (distinct s & d per window).
    cores = []
    nwin = np.zeros((n_cores, 4), np.int64)
    for c in range(n_cores):
        sl = slice(c * per, (c + 1) * per)
        sc, dc = s_all[sl], d_all[sl]
        a = (dc >= hb).astype(np.int64)
        b = (sc >= hb).astype(np.int64)
        seg = a * 2 + b
        per_seg = []
        for g in range(4):
            m = np.flatnonzero(seg == g)
            s_m, d_m = sc[m], dc[m]
            degd = np.bincount(d_m, minlength=N)
            degs = np.bincount(s_m, minlength=N)
            prio = np.argsort(-(degd[d_m] + degs[s_m]), kind="stable")
            win_p, nw = _window_assign(s_m[prio], d_m[prio], group, N)
            win = np.empty_like(win_p)
            win[prio] = win_p
            key = np.lexsort((s_m, win))
            per_seg.append((m[key], win[key], nw))
            nwin[c, g] = nw
        cores.append((sc, dc, efE_all[sl], efO_all[sl], per_seg))
    gchunk = group * unroll
    seg_nw = [int(_ceil(max(int(nwin[:, g].max()), 1) * group, gchunk)) // group
              for g in range(4)]
    seg_sz = [nw * group for nw in seg_nw]
    NP_ = int(sum(seg_sz))
    seg_start = [0, seg_sz[0], seg_sz[0] + seg_sz[1],
                 seg_sz[0] + seg_sz[1] + seg_sz[2]]

    shards = []
    for c in range(n_cores):
        sc, dc, efE_c, efO_c, per_seg = cores[c]
        sIdx = np.zeros(NP_, np.int64)
        dIdx = np.zeros(NP_, np.int64)
        ef2 = np.zeros((DE, 2 * NP_), BF)
        for g in range(4):
            a, b = g // 2, g % 2
            st = seg_start[g]
            s_tr = LO_TRASH if b == 0 else HI_TRASH
            d_tr = LO_TRASH if a == 0 else HI_TRASH
            sIdx[st:st + seg_sz[g]] = s_tr
            dIdx[st:st + seg_sz[g]] = d_tr
            order, wins, nw = per_seg[g]
            if order.size:
                counts = np.bincount(wins, minlength=nw)
                assert counts.max() <= group
                starts = np.concatenate(([0], np.cumsum(counts)))[:-1]
                rank = np.arange(order.size) - starts[wins]
                pos = st + wins * group + rank
                sIdx[pos] = sc[order] - hb * b
                dIdx[pos] = dc[order] - hb * a
                ef2[:, pos] = efE_c[order].T
                ef2[:, NP_ + pos] = efO_c[order].T
        assert sIdx.max() < 32768 and dIdx.max() < 32768

        # combined per-group idx layout: [s_g (128 cols) | d_g (128 cols)]
        ng = NP_ // group
        gsIdx = np.zeros((128, ng * 2 * (group // 16)), np.int16)
        w = group // 16
        for g in range(ng):
            gsIdx[:, g * 2 * w:(g * 2 + 1) * w] = \
                _wrap16(sIdx[g * group:(g + 1) * group])
            gsIdx[:, (g * 2 + 1) * w:(g * 2 + 2) * w] = \
                _wrap16(dIdx[g * group:(g + 1) * group])

        shards.append({
            "nfiL": nfiL, "nfiH": nfiH, "nffL": nffL, "nffH": nffH,
            "ef2": np.ascontiguousarray(ef2),
            "gsIdx": np.ascontiguousarray(gsIdx),
            "finIdx": fin_idx, "cu8": cu8,
        })

    meta = dict(N=N, NP=NP_, LOTAB=LOTAB, HITAB=HITAB, OUT_PAD=OUT_PAD,
                seg_sz=seg_sz, seg_start=seg_start, HB=hb,
                n_cores=n_cores, group=group, unroll=unroll,
                cu_zero=bool(not np.any(cu)))
    return shards, meta


def _build(meta):
    import os as _os

    import concourse.bass as bass
    import concourse.tile as tile
    from concourse import bacc, mybir

    f32 = mybir.dt.float32
    bf16 = mybir.dt.bfloat16
    i16 = mybir.dt.int16
    NP_ = meta["NP"]
    LOT = meta["LOTAB"]
    HIT = meta["HITAB"]
    OUT_PAD = meta["OUT_PAD"]
    group = meta["group"]
    unroll = meta["unroll"]
    n_cores = meta["n_cores"]
    hb = meta["HB"]
    jb = hb // CHUNK            # final-pass LO slab count
    GW = group // 16            # idx cols per half-group
    NBLK = group // 128         # 16 em blocks per group-half... per 2048 pairs
    NCH = 2 * group // CHUNK    # fm chunks per group (E cols + O cols)

    ABL = set(_os.environ.get("KV2_ABLATE", "").split(","))
    SEQIDX = _os.environ.get("KV2_SEQIDX") == "1"
    NQ = int(_os.environ.get("KV2_QUEUES", "1"))
    U_ENV = int(_os.environ.get("KV2_UNROLL", "64"))
    SBUF_SCAT = _os.environ.get("KV2_SBUF_SCAT", "1") == "1"
    GSP = _os.environ.get("KV2_GATHER_SP", "0") == "1"

    nc = bacc.Bacc("TRN2", target_bir_lowering=False, debug=False,
                   enable_asserts=False, num_devices=n_cores,
                   num_swdge_queues=NQ)

    nfiL_t = nc.dram_tensor("nfiL", [LOT, D], bf16, kind="ExternalInput")
    nfiH_t = nc.dram_tensor("nfiH", [HIT, D], bf16, kind="ExternalInput")
    nffL_t = nc.dram_tensor("nffL", [D, LOT], bf16, kind="ExternalInput")
    nffH_t = nc.dram_tensor("nffH", [D, HIT], bf16, kind="ExternalInput")
    ef2_t = nc.dram_tensor("ef2", [DE, 2 * NP_], bf16, kind="ExternalInput")
    gsIdx_t = nc.dram_tensor("gsIdx", [128, NP_ // 16 * 2], i16,
                             kind="ExternalInput")
    finIdx_t = nc.dram_tensor("finIdx", [128, LOT // 16], i16,
                              kind="ExternalInput")
    cu8_t = nc.dram_tensor("cu8", [128, 16 * D], bf16, kind="ExternalInput")
    WiA_t = nc.dram_tensor("WiA", [D, D], bf16, kind="ExternalInput")
    WiB_t = nc.dram_tensor("WiB", [DE, D], bf16, kind="ExternalInput")
    Wu_t = nc.dram_tensor("Wu", [D, D], bf16, kind="ExternalInput")
    WfA_t = nc.dram_tensor("WfA", [D, D], bf16, kind="ExternalInput")
    WfB_t = nc.dram_tensor("WfB", [D, D], bf16, kind="ExternalInput")
    id_t = nc.dram_tensor("ident", [D, D], bf16, kind="ExternalInput")
    out_t = nc.dram_tensor("out", [OUT_PAD, D], f32, kind="ExternalOutput")
    DBG = _os.environ.get("KV2_DEBUG_TAPS") == "1"
    if DBG:
        dbgL = [nc.dram_tensor(f"dbgL{p}", [LOT, D], bf16,
                               kind="ExternalOutput")
                for p in range(STEPS + 1)]
        dbgH = [nc.dram_tensor(f"dbgH{p}", [HIT, D], bf16,
                               kind="ExternalOutput")
                for p in range(STEPS + 1)]
        dbgh = [nc.dram_tensor(f"dbgh{k}", [D, NP_], bf16,
                               kind="ExternalOutput")
                for k in range(2)]

    with tile.TileContext(nc) as tc:
        with (
            tc.tile_pool(name="const", bufs=1) as constp,
            tc.tile_pool(name="work", bufs=2) as work,
            tc.tile_pool(name="scat", bufs=1) as scatp,
            tc.tile_pool(name="psF", bufs=3, space="PSUM") as psF,
            tc.tile_pool(name="psE", bufs=3, space="PSUM") as psE,
            tc.tile_pool(name="dram", bufs=1, space="DRAM") as dram,
        ):
            def const_load(name, shape, dt, src_ap):
                t = constp.tile(shape, dt, tag=name, name=name)
                nc.sync.dma_start(t[:], src_ap)
                return t

            WiA = const_load("WiA", [D, D], bf16, WiA_t.ap())
            WiB = const_load("WiB", [DE, D], bf16, WiB_t.ap())
            Wu_sb = const_load("Wu", [D, D], bf16, Wu_t.ap())
            WfA = const_load("WfA", [D, D], bf16, WfA_t.ap())
            WfB = const_load("WfB", [D, D], bf16, WfB_t.ap())
            id_sb = const_load("ident", [D, D], bf16, id_t.ap())
            cu8_sb = (None if meta["cu_zero"] and SBUF_SCAT else
                      const_load("cu8", [128, 16 * D], bf16, cu8_t.ap()))
            gsIdx_sb = constp.tile([128, NP_ // 16 * 2], i16, tag="gsIdx",
                                   name="gsIdx_sb")
            nc.sync.dma_start(gsIdx_sb[:], gsIdx_t.ap())
            finIdx_sb = constp.tile([128, LOT // 16], i16, tag="finIdx",
                                    name="finIdx_sb")
            nc.sync.dma_start(finIdx_sb[:], finIdx_t.ap())

            # ---- DRAM state ----
            hE = [dram.tile([D, NP_], bf16, name=f"hE{k}", tag=f"hE{k}")
                  for k in range(2)]
            hO = [dram.tile([D, NP_], bf16, name=f"hO{k}", tag=f"hO{k}")
                  for k in range(2)]
            LL = [dram.tile([LOT, D], bf16, name=f"aggLL{k}", tag=f"aggLL{k}")
                  for k in range(2)]
            LH = [dram.tile([HIT, D], bf16, name=f"aggLH{k}", tag=f"aggLH{k}")
                  for k in range(2)]
            SL = [dram.tile([LOT, D], bf16, name=f"aggSL{k}", tag=f"aggSL{k}",
                            addr_space="Shared") for k in range(STEPS + 1)]
            SH = [dram.tile([HIT, D], bf16, name=f"aggSH{k}", tag=f"aggSH{k}",
                            addr_space="Shared") for k in range(STEPS + 1)]
            # SBUF-resident local scatter tables (parity-split pairs)
            if SBUF_SCAT:
                sLO = [scatp.tile([128, LOT // 256, D], bf16, tag=f"sLO{t}",
                                  name=f"sLO{t}") for t in range(2)]
                sHI = [scatp.tile([128, HIT // 256, D], bf16, tag=f"sHI{t}",
                                  name=f"sHI{t}") for t in range(2)]
                s_tiles = (sLO, sHI)

            def fill_table(t, rows):
                if "fill" in ABL:
                    return
                zr = cu8_sb[:].rearrange("p (a f) -> p a f", f=D)
                r0 = 0
                while r0 < rows:
                    nc.sync.dma_start(
                        t[:][r0:r0 + 2048, :]
                        .rearrange("(a p) f -> p a f", p=128),
                        zr)
                    r0 += 2048

            def idx_cols(gg, which):
                # which: 0 = s half, 1 = d half, 2 = both
                if SEQIDX:  # timing experiment: sequential hot rows
                    return (finIdx_sb[:, 0:2 * GW] if which == 2
                            else finIdx_sb[:, 0:GW])
                if which == 2:
                    return gsIdx_sb[:, bass.ds(gg * 2 * GW, 2 * GW)]
                return gsIdx_sb[:, bass.ds((gg * 2 + which) * GW, GW)]

            qn = [0]

            def next_q():
                # gathers rotate across queues 1..NQ-1 (no mutual deps);
                # scatters stay on queue 0 (cross-queue WAW is unsafe)
                if NQ == 1:
                    return 0
                qn[0] = qn[0] % (NQ - 1) + 1
                return qn[0]

            def tab(t):
                return t[:] if hasattr(t, "opt") else t.ap()

            def body(kind, seg, gg, col0, gsrc, gtgt, hin, hout):
                """One group of `group` pairs.  gg = global group index,
                col0 = pair column base.  gsrc/gtgt: (LO, HI) table pairs."""
                a, b = seg // 2, seg % 2
                A = work.tile([128, 1, 2 * group], bf16, tag="A", name="A")
                if "gather" in ABL:
                    nc.vector.memset(A[:], 0.0)
                elif a == b:
                    nc.gpsimd.dma_gather(
                        A[:], tab(gsrc[b]), idx_cols(gg, 2),
                        num_idxs=2 * group, num_idxs_reg=2 * group,
                        elem_size=D, transpose=True, single_packet=GSP,
                        queue_num=next_q())
                else:
                    nc.gpsimd.dma_gather(
                        A[:, :, 0:group], tab(gsrc[b]), idx_cols(gg, 0),
                        num_idxs=group, num_idxs_reg=group,
                        elem_size=D, transpose=True, single_packet=GSP,
                        queue_num=next_q())
                    nc.gpsimd.dma_gather(
                        A[:, :, group:2 * group], tab(gsrc[a]), idx_cols(gg, 1),
                        num_idxs=group, num_idxs_reg=group,
                        elem_size=D, transpose=True, single_packet=GSP,
                        queue_num=next_q())
                Av = A[:].rearrange("p a n -> p (a n)")

                if kind == "step" and "h" in ABL:
                    kind = "init2"
                if kind == "step":
                    hld = work.tile([128, 2, group], bf16, tag="hld",
                                    name="hld", bufs=1)
                    nc.sync.dma_start(hld[:, 0, :],
                                      hin[1][:][:, bass.ds(col0, group)])
                    nc.sync.dma_start(hld[:, 1, :],
                                      hin[0][:][:, bass.ds(col0, group)])
                elif kind == "init":
                    ef_sb = work.tile([DE, 2 * group], bf16, tag="ef",
                                      name="ef_sb", bufs=1)
                    nc.sync.dma_start(ef_sb[:, 0:group],
                                      ef2_t.ap()[:, bass.ds(col0, group)])
                    nc.sync.dma_start(
                        ef_sb[:, group:2 * group],
                        ef2_t.ap()[:, bass.ds(NP_ + col0, group)])

                emE = work.tile([128, NBLK, D], bf16, tag="emE", name="emE")
                emO = work.tile([128, NBLK, D], bf16, tag="emO", name="emO")
                for c in range(NCH):
                    c0 = c * CHUNK
                    cg = c0 if c < NCH // 2 else c0 - group
                    ps = psF.tile([128, CHUNK], f32, tag="fm", name="ps")
                    hc = work.tile([128, CHUNK], bf16, tag="hc", name="hc",
                                   bufs=4)
                    if kind == "step":
                        other, self_ = (0, 1) if c < NCH // 2 else (1, 0)
                        mc = work.tile([128, CHUNK], bf16, tag="mc",
                                       name="mc", bufs=4)
                        nc.vector.tensor_sub(mc[:], Av[:, c0:c0 + CHUNK],
                                             hld[:, other, cg:cg + CHUNK])
                        nc.tensor.matmul(ps[:], Wu_sb[:], mc[:],
                                         start=True, stop=True)
                        tmp = work.tile([128, CHUNK], bf16, tag="tmp",
                                        name="tmp", bufs=4)
                        nc.vector.tensor_add(tmp[:], ps[:],
                                             hld[:, self_, cg:cg + CHUNK])
                        nc.vector.tensor_relu(hc[:], tmp[:])
                    elif kind == "init2":
                        nc.tensor.matmul(ps[:], Wu_sb[:], Av[:, c0:c0 + CHUNK],
                                         start=True, stop=True)
                        nc.vector.tensor_relu(hc[:], ps[:])
                    else:
                        nc.tensor.matmul(ps[:], WiA[:], Av[:, c0:c0 + CHUNK],
                                         start=True, stop=False)
                        nc.tensor.matmul(ps[:], WiB[:],
                                         ef_sb[:, c0:c0 + CHUNK],
                                         start=False, stop=True)
                        nc.vector.tensor_relu(hc[:], ps[:])
                    if hout is not None and "h" not in ABL:
                        nc.sync.dma_start(
                            hout[0 if c < NCH // 2 else 1][:]
                            [:, bass.ds(col0 + cg, CHUNK)], hc[:])
                    pse = psE.tile([128, CHUNK // D, D], bf16, tag="em",
                                   name="pse")
                    for t in range(CHUNK // D):
                        nc.tensor.transpose(
                            pse[:, t, :],
                            hc[:, t * D:(t + 1) * D], id_sb[:])
                    em_t = emE if c < NCH // 2 else emO
                    eb = cg // D
                    nc.scalar.copy(em_t[:, eb:eb + CHUNK // D, :], pse[:])
                # E states scatter by d into half `a`; O states by s into `b`
                if "scatter" in ABL:
                    pass
                elif SBUF_SCAT:
                    nc.gpsimd.dma_scatter_add(
                        s_tiles[a][0][:], emE[:], idx_cols(gg, 1),
                        num_idxs=group, num_idxs_reg=group, elem_size=D,
                        single_packet=False, queue_num=0,
                        sbuf_tokens_per_rank=128, parity_reg=0,
                        out_ap_other=s_tiles[a][1][:])
                    nc.gpsimd.dma_scatter_add(
                        s_tiles[b][0][:], emO[:], idx_cols(gg, 0),
                        num_idxs=group, num_idxs_reg=group, elem_size=D,
                        single_packet=False, queue_num=0,
                        sbuf_tokens_per_rank=128, parity_reg=0,
                        out_ap_other=s_tiles[b][1][:])
                else:
                    nc.gpsimd.dma_scatter_add(
                        gtgt[a][:], emE[:], idx_cols(gg, 1),
                        num_idxs=group, num_idxs_reg=group, elem_size=D,
                        single_packet=False, queue_num=0)
                    nc.gpsimd.dma_scatter_add(
                        gtgt[b][:], emO[:], idx_cols(gg, 0),
                        num_idxs=group, num_idxs_reg=group, elem_size=D,
                        single_packet=False, queue_num=0)

            def loop_i(lo, hi, step, body_fn):
                if _os.environ.get("BASS_SIM_UNROLL") == "1":
                    for i in range(lo, hi, step):
                        body_fn(i)
                else:
                    with tc.For_i(lo, hi, step) as i:
                        body_fn(i)

            def allreduce(src, dst):
                if "ar" in ABL:
                    return
                nc.gpsimd.collective_compute(
                    "AllReduce", mybir.AluOpType.add,
                    replica_groups=[list(range(n_cores))],
                    ins=[src.opt()], outs=[dst.opt()])

            CU_ZERO = meta["cu_zero"]

            def dump_half(tiles, Ltab):
                v = Ltab[:].rearrange("(g tp) f -> tp g f", tp=256)
                if CU_ZERO:
                    nc.sync.dma_start(v[0:128], tiles[0][:])
                    nc.sync.dma_start(v[128:256], tiles[1][:])
                else:
                    nc.gpsimd.dma_start(v[0:128], tiles[0][:],
                                        accum_op=mybir.AluOpType.add)
                    nc.gpsimd.dma_start(v[128:256], tiles[1][:],
                                        accum_op=mybir.AluOpType.add)

            def run_pass(p):
                kind = "init" if p == 0 else "step"
                par = p % 2
                gsrc = (SL[p - 1], SH[p - 1]) if p > 0 else (nfiL_t, nfiH_t)
                gtgt = (LL[par], LH[par])
                hin = (hE[p % 2], hO[p % 2]) if p > 0 else None
                hout = ((hE[(p + 1) % 2], hO[(p + 1) % 2])
                        if p < STEPS else None)
                if SBUF_SCAT:
                    if not CU_ZERO:  # bias lands via accumulate-dump
                        fill_table(LL[par], LOT)
                        fill_table(LH[par], HIT)
                    for t in sLO + sHI:
                        nc.vector.memset(t[:], 0.0)
                elif p < STEPS:   # zero next pass's scatter targets
                    fill_table(LL[1 - par], LOT)
                    fill_table(LH[1 - par], HIT)
                for seg in range(4):
                    n_groups = meta["seg_sz"][seg] // group
                    g0 = meta["seg_start"][seg] // group
                    u = U_ENV
                    n_loop = n_groups // u * u

                    def emit(i, seg=seg, g0=g0):
                        gg = g0 + i
                        body(kind, seg, gg, gg * group, gsrc, gtgt, hin, hout)

                    if n_loop:
                        loop_i(0, n_loop, u,
                               lambda i: [emit(i + j) for j in range(u)])
                    for r in range(n_loop, n_groups):
                        emit(r)
                    if seg == 2:
                        if SBUF_SCAT:
                            dump_half(sLO, LL[par])
                        allreduce(LL[par], SL[p])
                if SBUF_SCAT:
                    dump_half(sHI, LH[par])
                allreduce(LH[par], SH[p])
                if DBG:
                    nc.sync.dma_start(dbgL[p].ap(), SL[p][:])
                    nc.sync.dma_start(dbgH[p].ap(), SH[p][:])
                    if p == 0:
                        nc.sync.dma_start(dbgh[0].ap(), hE[1][:])
                        nc.sync.dma_start(dbgh[1].ap(), hO[1][:])

            if not SBUF_SCAT:
                fill_table(LL[0], LOT)
                fill_table(LH[0], HIT)
            for p in range(STEPS + 1):
                run_pass(p)

            # ---- final: out = relu([nf || agg] @ Wf + bf) ----------------
            aggF = (SL[STEPS], SH[STEPS])
            nff = (nffL_t, nffH_t)

            def final_body(j, hi, roff):
                nf_fm = work.tile([128, CHUNK], bf16, tag="fnf", name="nf_fm")
                nc.sync.dma_start(
                    nf_fm[:], nff[hi].ap()[:, bass.ds(j * CHUNK - roff, CHUNK)])
                ag_fm = work.tile([128, 1, CHUNK], bf16, tag="fag",
                                  name="ag_fm")
                nc.gpsimd.dma_gather(
                    ag_fm[:], aggF[hi][:],
                    finIdx_sb[:, bass.ds(j * (CHUNK // 16) - roff // 16,
                                         CHUNK // 16)],
                    num_idxs=CHUNK, num_idxs_reg=CHUNK, elem_size=D,
                    transpose=True, single_packet=GSP)
                pso = psE.tile([128, CHUNK // D, D], f32, tag="em",
                               name="pso")
                for t in range(CHUNK // D):
                    nc.tensor.matmul(pso[:, t, :],
                                     nf_fm[:, t * D:(t + 1) * D],
                                     WfA[:], start=True, stop=False)
                    nc.tensor.matmul(pso[:, t, :],
                                     ag_fm[:, 0, t * D:(t + 1) * D],
                                     WfB[:], start=False, stop=True)
                o_t = work.tile([128, CHUNK // D, D], f32, tag="fot",
                                name="o_t")
                nc.scalar.activation(o_t[:], pso[:],
                                     mybir.ActivationFunctionType.Relu)
                nc.sync.dma_start(
                    out_t.ap()[bass.ds(j * CHUNK, CHUNK), :]
                    .rearrange("(a p) f -> p a f", p=128),
                    o_t[:])

            def unrolled(lo, hi, u, fn):
                n_loop = (hi - lo) // u * u
                if n_loop:
                    loop_i(lo, lo + n_loop, u,
                           lambda j: [fn(j + k) for k in range(u)])
                for r in range(lo + n_loop, hi):
                    fn(r)

            unrolled(0, jb, 7, lambda j: final_body(j, 0, 0))
            unrolled(jb, OUT_PAD // CHUNK, 7, lambda j: final_body(j, 1, hb))

    nc.compile()
    return nc


LAST_RESULTS = None
LAST_TIMES = None


def _run_spmd(nc, in_maps, time_iters=0):
    """Execute the bass module on len(in_maps) axon cores via PJRT."""
    import time as _time

    import jax
    from jax.experimental.shard_map import shard_map
    from jax.sharding import Mesh, NamedSharding, PartitionSpec

    from concourse import bass2jax, mybir

    bass2jax.install_neuronx_cc_hook()
    n_cores = len(in_maps)
    partition_name = (nc.partition_id_tensor.name
                      if nc.partition_id_tensor else None)
    in_names, out_names, out_avals, zero_outs = [], [], [], []
    for alloc in nc.m.functions[0].allocations:
        if not isinstance(alloc, mybir.MemoryLocationSet):
            continue
        name = alloc.memorylocations[0].name
        if alloc.kind == "ExternalInput":
            if name != partition_name:
                in_names.append(name)
        elif alloc.kind == "ExternalOutput":
            shape = tuple(alloc.tensor_shape)
            dtype = mybir.dt.np(alloc.dtype)
            out_names.append(name)
            out_avals.append(jax.core.ShapedArray(shape, dtype))
            zero_outs.append(np.zeros(shape, dtype))
    n_params = len(in_names)
    full_in_names = list(in_names) + list(out_names)
    if partition_name is not None:
        full_in_names.append(partition_name)

    def _body(*args):
        operands = list(args)
        if partition_name is not None:
            operands.append(bass2jax.partition_id_tensor())
        outs = bass2jax._bass_exec_p.bind(
            *operands,
            out_avals=tuple(out_avals),
            in_names=tuple(full_in_names),
            out_names=tuple(out_names),
            lowering_input_output_aliases=(),
            sim_require_finite=True,
            sim_require_nnan=True,
            nc=nc,
        )
        return tuple(outs)

    devices = jax.devices()[:n_cores]
    mesh = Mesh(np.asarray(devices), ("core",))
    spec = NamedSharding(mesh, PartitionSpec("core"))
    n_in = n_params + len(zero_outs)
    fn = jax.jit(shard_map(_body, mesh=mesh,
                           in_specs=(PartitionSpec("core"),) * n_in,
                           out_specs=(PartitionSpec("core"),) * len(out_names),
                           check_rep=False))
    dev_in = [
        jax.device_put(
            np.concatenate([np.asarray(in_maps[c][k]) for c in range(n_cores)], 0),
            spec)
        for k in in_names
    ]
    dev_zero = [
        jax.device_put(np.zeros((n_cores * z.shape[0], *z.shape[1:]), z.dtype), spec)
        for z in zero_outs
    ]
    out = fn(*dev_in, *dev_zero)
    jax.block_until_ready(out)
    times = []
    for _ in range(time_iters):
        t0 = _time.perf_counter()
        out2 = fn(*dev_in, *dev_zero)
        jax.block_until_ready(out2)
        times.append(_time.perf_counter() - t0)
    results = [
        {name: np.asarray(out[i]).reshape(n_cores, *out_avals[i].shape)[c]
         for i, name in enumerate(out_names)}
        for c in range(n_cores)
    ]
    return results, times


def kernel(node_feature, edge_feature, edge_src, edge_dst,
           Wi, bi, Wu, bu, Wf, bf):
    import os

    import ml_dtypes

    BF = ml_dtypes.bfloat16
    global LAST_RESULTS, LAST_TIMES
    shards, meta = _prep(node_feature, edge_feature, edge_src, edge_dst,
                         Wi, bi, Wu, bu, Wf, bf)
    nc = _build(meta)

    Wi = np.asarray(Wi, np.float32)
    common = {
        "WiA": Wi[:D].astype(BF), "WiB": Wi[D:].astype(BF),
        "Wu": np.asarray(Wu, np.float32).astype(BF),
        "WfA": np.asarray(Wf, np.float32)[:D].astype(BF),
        "WfB": np.asarray(Wf, np.float32)[D:].astype(BF),
        "ident": np.eye(D, dtype=np.float32).astype(BF),
    }
    in_maps = [dict(sh, **common) for sh in shards]
    time_iters = int(os.environ.get("KERNEL_TIME_ITERS", "0"))
    results, times = _run_spmd(nc, in_maps, time_iters=time_iters)
    LAST_RESULTS = results
    LAST_TIMES = times
    return np.asarray(results[0]["out"][:meta["N"]], np.float32)
